# revision 50
# baseline (speedup 1.0000x reference)
"""Trainium2 Bass kernel for AxialAttention (attention along W axis).

Sharding: pure data-parallel over (B=4) x (H split in 2) = 8 shards, one
per NeuronCore. Attention mixes pixels only along W within a single
(b, head, h-row), so splitting H requires no collectives.

Per-core structure (shard = [C=512, 48 rows x 96 cols]), organized as a
uniform pipeline over 48 row-slots (1 slot = one 96-pixel attention row):

  slot s (row i of group t, groups = 4 rows = 384 pixels):
    1. v projection for row s (pixel-major out [96pix, 512ch])
    2. scores^T = k^T.T @ q^T per head -> [j, i] blocks in PSUM,
       exp via ACT (scale=0.125, no max subtraction: |s*scale| < 7)
    3. two q/k-projection chunks of group t+1 (spread over the 4 slots)
    4. AV for row s-1 in transposed form: lhsT = expS [j,i], rhs = v
       [j, d] -> out [i-pixel, d-channel], ap=64/head (vs 96 for the
       [d, i] form); softmax sums via ap-1 matmuls (rhs = ones[96,1])
    5. normalize on DVE with broadcast reciprocal -> a [96, 512] bf16
    6. PE-transpose a back to channel-major (4 x [96,128] -> [128,96])
    7. one out-projection row (s-3) + bias via ACT -> y staging
  PSUM->SBUF evacuations are spread across DVE and ACT (GPSIMD cannot
  touch PSUM); PE is the only near-saturated engine (~95%).
"""

import numpy as np
import ml_dtypes

import concourse.bass as bass
import concourse.tile as tile
from concourse import mybir

BF16 = mybir.dt.bfloat16
F32 = mybir.dt.float32

B, C, H, W = 4, 512, 96, 96
HEADS, D = 8, 64
SCALE = 0.125
NCORES = 8
RPC = H // 2          # 48 rows per core
PIX = RPC * W         # 4608 pixels per core
GRP = 12              # pixel groups
GPIX = PIX // GRP     # 384 pixels per group = 4 rows
NROW = RPC            # 48 row-slots

# diagnostic hook: current emission phase (used by tools/, no runtime effect)
PHASE = [None]

# head processing order: evens (K-offset 0) then odds (K-offset 64) so
# concurrent different-K-offset score matmuls never share a PSUM bank
HEAD_ORDER = [0, 2, 4, 6, 1, 3, 5, 7]
# scores block of order-position idx lives at column 128*idx in the
# [96, 1024] scores tile (keeps each 96-col block inside one 2KB bank)
SCORE_COL = [128 * i for i in range(8)]
SUMS_COL = 992  # sums for real head h at column SUMS_COL + h (bank B)


def build_nc(apply_waitfix=True):
    nc = bass.Bass(trn_type="TRN2")
    x_d = nc.declare_dram_parameter("x", [4, 128, PIX], BF16, isOutput=False)
    wqk_d = nc.declare_dram_parameter("wqk", [4, 128, 1024], BF16, isOutput=False)
    wv_d = nc.declare_dram_parameter("wv", [4, 128, 512], BF16, isOutput=False)
    wo_d = nc.declare_dram_parameter("wo", [4, 128, 512], BF16, isOutput=False)
    bias_d = nc.declare_dram_parameter("bias", [4, 128, 1], F32, isOutput=False)
    ident_d = nc.declare_dram_parameter("ident", [96, 96], BF16, isOutput=False)
    y_d = nc.declare_dram_parameter("y", [512, PIX], F32, isOutput=True)

    with tile.TileContext(nc) as tc:
        with (
            tc.tile_pool(name="persist", bufs=1) as persist,
            tc.tile_pool(name="vrow", bufs=4) as vrow,
            tc.tile_pool(name="erow", bufs=4) as erow,
            tc.tile_pool(name="arow", bufs=4) as arow,
            tc.tile_pool(name="rrow", bufs=4) as rrow,
            tc.tile_pool(name="ostage", bufs=2) as ostage,
            tc.tile_pool(name="psA", bufs=2, space="PSUM") as psA,
            tc.tile_pool(name="psS", bufs=1, space="PSUM") as psS,
            tc.tile_pool(name="psV", bufs=1, space="PSUM") as psV,
            tc.tile_pool(name="psT", bufs=1, space="PSUM") as psT,
            tc.tile_pool(name="psSum", bufs=1, space="PSUM") as psSum,
            tc.tile_pool(name="psO", bufs=1, space="PSUM") as psO,
        ):
            # --- PE warmup: dependency-free dummy matmuls fill the
            # initial DMA wait and finish the clock ramp -----------------
            warm_sb = persist.tile([128, 512], BF16, tag="warm")
            nc.vector.memset(warm_sb[:, 0:128], 0.0)
            wps = psA.tile([128, 512], F32, tag="psA")
            for _ in range(4):
                nc.tensor.matmul(wps[:, 0:128], lhsT=warm_sb[:, 0:128],
                                 rhs=warm_sb[:, 0:128])
            nc.vector.memset(warm_sb[:, 128:512], 0.0)
            for _ in range(9):
                nc.tensor.matmul(wps[:, 0:512], lhsT=warm_sb[:, 0:128],
                                 rhs=warm_sb[:, :])

            # --- persistent loads (wqk + first x tiles first so the
            # projection GEMMs start as early as possible) --------------
            wqk_t, wv_t, wo_t, bias_t = [], [], [], []
            for cc in range(4):
                wt = persist.tile([128, 1024], BF16, tag=f"wqk{cc}")
                nc.gpsimd.dma_start(out=wt[:, :], in_=wqk_d[cc])
                wqk_t.append(wt)
            # x in growing chunks: early groups land fast, later ones
            # amortize the ~0.65us per-DMA sequencer dispatch cost
            CHUNKS = [(0, 1), (1, 2), (2, 4), (4, 6), (6, 9), (9, 12)]
            x_t = [[None] * GRP for _ in range(4)]
            for ci, (t0, t1) in enumerate(CHUNKS):
                w = (t1 - t0) * GPIX
                for cc in range(4):
                    xt = persist.tile([128, w], BF16, tag=f"x{cc}_c{ci}")
                    nc.sync.dma_start(
                        out=xt[:, :],
                        in_=x_d[cc][:, t0 * GPIX:t1 * GPIX])
                    for t in range(t0, t1):
                        x_t[cc][t] = xt[:, (t - t0) * GPIX:(t - t0 + 1) * GPIX]
                if ci == 0:
                    ones_t = persist.tile([96, 1], BF16, tag="ones")
                    nc.vector.memset(ones_t[:, :], 1.0)
                elif ci == 1:
                    for cc in range(4):
                        vt = persist.tile([128, 512], BF16, tag=f"wv{cc}")
                        nc.sync.dma_start(out=vt[:, :], in_=wv_d[cc])
                        wv_t.append(vt)
                    ident_t = persist.tile([96, 96], BF16, tag="ident")
                    nc.sync.dma_start(out=ident_t[:, :], in_=ident_d[:, :])
                elif ci == 2:
                    for cc in range(4):
                        ot = persist.tile([128, 512], BF16, tag=f"wo{cc}")
                        nc.sync.dma_start(out=ot[:, :], in_=wo_d[cc])
                        wo_t.append(ot)
                        bt = persist.tile([128, 1], F32, tag=f"bias{cc}")
                        nc.sync.dma_start(out=bt[:, :], in_=bias_d[cc])
                        bias_t.append(bt)

            qk_t = [[None] * GRP for _ in range(8)]   # [oc][t] bf16 [128,384]
            # per-row state tiles, indexed by slot
            v_sb = [None] * NROW      # [96, 512] bf16 pixel-major v
            e_sb = [None] * NROW      # [96, 768] bf16 expS (head-order-major)
            av_ps = [None] * NROW     # AV psum tile [96, 512]
            a_sb = [None] * NROW      # [96, 512] bf16 normalized attn out
            rc_sb = [None] * NROW     # [96, 8] f32 reciprocal sums
            a_cm = [None] * GRP       # [128, 4, 384] bf16 channel-major attn
            o_sb = [None] * GRP       # y staging [128, 4, 384] f32 per group

            def emit_qk_chunk(t, oc, pool=None, ptag="psA"):
                """One q/k projection output chunk [128, 384] for group t."""
                PHASE[0] = f"qk(t{t},oc{oc})"
                if pool is None:
                    pool = psA
                qps = pool.tile([128, GPIX], F32, tag=ptag)
                for cc in range(4):
                    nc.tensor.matmul(
                        qps[:, 0:GPIX],
                        lhsT=wqk_t[cc][:, oc * 128:(oc + 1) * 128],
                        rhs=x_t[cc][t][:, :],
                        start=(cc == 0), stop=(cc == 3),
                    )
                qt = persist.tile([128, GPIX], BF16, tag=f"qk{oc}_{t}")
                if oc % 2 == 0:
                    nc.scalar.copy(out=qt[:, :], in_=qps[:, 0:GPIX])
                else:
                    nc.vector.tensor_copy(out=qt[:, :], in_=qps[:, 0:GPIX])
                qk_t[oc][t] = qt

            def emit_front(s, scores_first=False):
                """v projection + scores + exp for row-slot s."""
                PHASE[0] = f"front({s})"
                t, rr = s // 4, s % 4
                rsl = slice(rr * 96, rr * 96 + 96)

                def do_vproj():
                    vps = psA.tile([128, 512], F32, tag="psA")
                    for cc in range(4):
                        nc.tensor.matmul(
                            vps[0:96, 0:512],
                            lhsT=x_t[cc][t][:, rsl],
                            rhs=wv_t[cc][:, :],
                            start=(cc == 0), stop=(cc == 3),
                        )
                    v_sb[s] = vrow.tile([96, 512], BF16, name="vsb", tag="vsb")
                    nc.vector.tensor_copy(out=v_sb[s][:, :],
                                          in_=vps[0:96, 0:512])

                def do_scores():
                    # scores^T per head: out[j, i] block at col 128*idx
                    sps = psS.tile([96, 1024], F32, tag="psS")
                    for idx, h in enumerate(HEAD_ORDER):
                        qc, half = h // 2, 64 * (h % 2)
                        col = SCORE_COL[idx]
                        nc.tensor.matmul(
                            sps[0:96, col:col + 96],
                            lhsT=qk_t[4 + qc][t][half:half + 64, rsl],
                            rhs=qk_t[qc][t][half:half + 64, rsl],
                        )
                    e_sb[s] = erow.tile([96, 768], BF16, name="esb", tag="esb")
                    nc.scalar.activation(
                        out=e_sb[s].rearrange("p (k n) -> p k n", k=8),
                        in_=sps[0:96, :].rearrange("p (k m) -> p k m", k=8)
                            [:, :, 0:96],
                        func=mybir.ActivationFunctionType.Exp,
                        scale=SCALE,
                    )

                if scores_first:
                    do_scores()
                    do_vproj()
                else:
                    do_vproj()
                    do_scores()

            def emit_mid(s):
                """AV (transposed form) + sums + reciprocal for slot s."""
                PHASE[0] = f"mid({s})"
                avp = psV.tile([96, 512], F32, tag="psV")
                av_ps[s] = avp
                smp = psSum.tile([96, 8], F32, tag="psSum")
                for idx, h in enumerate(HEAD_ORDER):
                    eh = e_sb[s][:, idx * 96:(idx + 1) * 96]
                    nc.tensor.matmul(
                        avp[0:96, h * 64:(h + 1) * 64],
                        lhsT=eh,
                        rhs=v_sb[s][:, h * 64:(h + 1) * 64],
                    )
                    nc.tensor.matmul(
                        smp[:, h:h + 1],
                        lhsT=eh,
                        rhs=ones_t[:, :],
                    )
                rc_sb[s] = rrow.tile([96, 8], F32, name="rcsb", tag="rcsb")
                nc.vector.reciprocal(out=rc_sb[s][:, :], in_=smp[:, :])

            def emit_back_norm(s):
                """normalize attention output for slot s (DVE only)."""
                PHASE[0] = f"back({s})"
                a_sb[s] = arow.tile([96, 512], BF16, name="asb", tag="asb")
                nc.vector.tensor_tensor(
                    out=a_sb[s].rearrange("p (h d) -> p h d", h=8),
                    in0=av_ps[s][0:96, :].rearrange("p (h d) -> p h d", h=8),
                    in1=rc_sb[s][:, :, None].broadcast_to([96, 8, 64]),
                    op=mybir.AluOpType.mult,
                )

            def emit_back_tp(s):
                """PE transpose to channel-major + evacuate for slot s."""
                PHASE[0] = f"back({s})"
                t, rr = s // 4, s % 4
                tp = psT.tile([128, 384], BF16, tag="psT")
                for cc in range(4):
                    nc.tensor.transpose(
                        out=tp[:, cc * 96:(cc + 1) * 96],
                        in_=a_sb[s][:, cc * 128:(cc + 1) * 128],
                        identity=ident_t[:, :],
                    )
                if a_cm[t] is None:
                    a_cm[t] = persist.tile([128, 4, GPIX], BF16, name=f"acm{t}", tag=f"acm{t}")
                nc.vector.tensor_copy(
                    out=a_cm[t][:, :, rr * 96:rr * 96 + 96],
                    in_=tp[:, :].rearrange("p (c n) -> p c n", c=4),
                )

            def emit_outproj_row(s):
                """out projection + bias for the 96 pixels of row-slot s."""
                PHASE[0] = f"outproj({s})"
                t, rr = s // 4, s % 4
                # epilogue rows: use the otherwise-idle psA pool so
                # consecutive tail rows don't serialize on psO release
                pool = psA if s >= NROW - 3 else psO
                ops_ = pool.tile([128, 384], F32, tag="psA" if s >= NROW - 3 else "psO")
                for oc in range(4):
                    for cc in range(4):
                        nc.tensor.matmul(
                            ops_[:, oc * 96:oc * 96 + 96],
                            lhsT=wo_t[cc][:, oc * 128:(oc + 1) * 128],
                            rhs=a_cm[t][:, cc, rr * 96:rr * 96 + 96],
                            start=(cc == 0), stop=(cc == 3),
                        )
                tail = t == GRP - 1
                if o_sb[t] is None:
                    o_sb[t] = ostage.tile([128, 4, GPIX], F32, name="osb", tag="osb")
                for oc in range(4):
                    dst = o_sb[t][:, oc, rr * 96:rr * 96 + 96]
                    src = ops_[:, oc * 96:oc * 96 + 96]
                    if tail and oc in (1, 3):
                        # tail: spread biases across ACT+DVE so psum frees fast
                        nc.vector.tensor_scalar_add(dst, src, bias_t[oc][:, :])
                    else:
                        nc.scalar.add(out=dst, in_=src, add=bias_t[oc][:, :])
                if rr == 3 and t < GRP - 2:
                    # one DMA per group: a single HWDGE dispatch covers all
                    # four output-channel blocks
                    nc.sync.dma_start(
                        out=y_d[0:512, t * GPIX:(t + 1) * GPIX]
                            .rearrange("(c p) n -> p c n", c=4),
                        in_=o_sb[t][:, :, :])
                elif rr == 3:
                    # last group: per-oc DMAs; DVE-biased blocks are ready
                    # first, so dispatch them first on the serial HWDGE
                    for oc in (1, 3, 0, 2):
                        nc.sync.dma_start(
                            out=y_d[oc * 128:(oc + 1) * 128,
                                    t * GPIX:(t + 1) * GPIX],
                            in_=o_sb[t][:, oc, :])

            # --- prologue: q/k projection for group 0 (rotate across the
            # idle psum pools so evacuation latency never gates the PE) ---
            _ppools = [(psA, "psA"), (psO, "psO"), (psV, "psV"),
                       (psA, "psA"), (psO, "psO"), (psV, "psV"),
                       (psA, "psA"), (psO, "psO")]
            for oc in range(8):
                emit_qk_chunk(0, oc, pool=_ppools[oc][0], ptag=_ppools[oc][1])
            # --- uniform row-slot pipeline ------------------------------
            for s in range(NROW):
                t, i = s // 4, s % 4
                if s >= 2:
                    emit_back_norm(s - 2)
                emit_front(s, scores_first=(s == NROW - 1))
                if s >= 1:
                    emit_mid(s - 1)
                if s >= 2:
                    emit_back_tp(s - 2)
                if t + 1 < GRP:
                    emit_qk_chunk(t + 1, 2 * i)
                    emit_qk_chunk(t + 1, 2 * i + 1)
                if s >= 3:
                    emit_outproj_row(s - 3)
            # --- epilogue: tightest drain order -------------------------
            emit_back_norm(NROW - 2)
            emit_mid(NROW - 1)
            emit_back_tp(NROW - 2)
            emit_outproj_row(NROW - 3)
            emit_back_norm(NROW - 1)
            emit_back_tp(NROW - 1)
            emit_outproj_row(NROW - 2)
            emit_outproj_row(NROW - 1)

    if apply_waitfix:
        split_excess_waits(nc)
    return nc


# --- walrus workaround -------------------------------------------------
# The walrus build in this container rejects instructions carrying more
# than a small number of semaphore waits (1 for CTRL-queue NoOp/Drain).
# TileContext's exit drain can exceed that. Split: keep at most one wait
# on the original instruction and insert same-engine NoOps immediately
# before it, each carrying one of the excess waits.
def split_excess_waits(nc):
    import bass_rust
    n_split = 0
    for f in nc.m.functions:
        for blk in f.blocks:
            newlist = []
            changed = False
            for inst in blk.instructions:
                si = inst.sync_info
                w = list(si.on_wait) if si is not None else []
                if len(w) > 1:
                    *pre, last = w
                    for ci, wait in enumerate(pre):
                        nop = mybir.InstNoOp(
                            name=f"{inst.name}-wsplit{ci}", ins=[], outs=[])
                        nop.engine = inst.engine
                        nop.sync_info = bass_rust.SyncInfo(
                            on_update=[], on_wait=[wait])
                        newlist.append(nop)
                    inst.sync_info.on_wait = [last]
                    changed = True
                    n_split += 1
                newlist.append(inst)
            if changed:
                blk.instructions = newlist
    return n_split


def shard_inputs(x, w_qkv, w_out, b_out):
    """Full inputs -> list of 8 per-core input maps."""
    x = np.asarray(x, dtype=np.float32)
    w_qkv = np.asarray(w_qkv, dtype=np.float32)
    w_out = np.asarray(w_out, dtype=np.float32)
    b_out = np.asarray(b_out, dtype=np.float32)
    bf = ml_dtypes.bfloat16
    wqk = np.ascontiguousarray(w_qkv[:1024].T).astype(bf).reshape(4, 128, 1024)
    wv = np.ascontiguousarray(w_qkv[1024:].T).astype(bf).reshape(4, 128, 512)
    wo = np.ascontiguousarray(w_out.T).astype(bf).reshape(4, 128, 512)
    bias = b_out.astype(np.float32).reshape(4, 128, 1)
    ident = np.eye(96, dtype=bf)
    xb = x.astype(bf)  # [4, 512, 96, 96]
    in_maps = []
    for core in range(NCORES):
        b, half = core // 2, core % 2
        xs = np.ascontiguousarray(
            xb[b, :, half * RPC:(half + 1) * RPC, :]).reshape(4, 128, PIX)
        in_maps.append({"x": xs, "wqk": wqk, "wv": wv, "wo": wo,
                        "bias": bias, "ident": ident})
    return in_maps


def unshard_outputs(results):
    out = np.empty((B, C, H, W), np.float32)
    for core in range(NCORES):
        b, half = core // 2, core % 2
        out[core // 2, :, (core % 2) * RPC:((core % 2) + 1) * RPC, :] = (
            results[core]["y"].reshape(C, RPC, W))
    return out


_NC_CACHE = None


def kernel(x, w_qkv, w_out, b_out):
    global _NC_CACHE
    from concourse.bass_utils import run_bass_kernel_spmd
    if _NC_CACHE is None:
        _NC_CACHE = build_nc()
    in_maps = shard_inputs(x, w_qkv, w_out, b_out)
    res = run_bass_kernel_spmd(_NC_CACHE, in_maps, list(range(NCORES)))
    return unshard_outputs(res.results)


# revision 58
# speedup vs baseline: 1.0032x; 1.0032x over previous
"""Trainium2 Bass kernel for AxialAttention (attention along W axis).

Sharding: pure data-parallel over (B=4) x (H split in 2) = 8 shards, one
per NeuronCore. Attention mixes pixels only along W within a single
(b, head, h-row), so splitting H requires no collectives.

Per-core structure (shard = [C=512, 48 rows x 96 cols]), organized as a
uniform pipeline over 48 row-slots (1 slot = one 96-pixel attention row):

  slot s (row i of group t, groups = 4 rows = 384 pixels):
    1. v projection for row s (pixel-major out [96pix, 512ch])
    2. scores^T = k^T.T @ q^T per head -> [j, i] blocks in PSUM,
       exp via ACT (scale=0.125, no max subtraction: |s*scale| < 7)
    3. two q/k-projection chunks of group t+1 (spread over the 4 slots)
    4. AV for row s-1 in transposed form: lhsT = expS [j,i], rhs = v
       [j, d] -> out [i-pixel, d-channel], ap=64/head (vs 96 for the
       [d, i] form); softmax sums via ap-1 matmuls (rhs = ones[96,1])
    5. normalize on DVE with broadcast reciprocal -> a [96, 512] bf16
    6. PE-transpose a back to channel-major (4 x [96,128] -> [128,96])
    7. one out-projection row (s-3) + bias via ACT -> y staging
  PSUM->SBUF evacuations are spread across DVE and ACT (GPSIMD cannot
  touch PSUM); PE is the only near-saturated engine (~95%).
"""

import numpy as np
import ml_dtypes

import concourse.bass as bass
import concourse.tile as tile
from concourse import mybir

BF16 = mybir.dt.bfloat16
F32 = mybir.dt.float32

B, C, H, W = 4, 512, 96, 96
HEADS, D = 8, 64
SCALE = 0.125
NCORES = 8
RPC = H // 2          # 48 rows per core
PIX = RPC * W         # 4608 pixels per core
GRP = 12              # pixel groups
GPIX = PIX // GRP     # 384 pixels per group = 4 rows
NROW = RPC            # 48 row-slots

# diagnostic hook: current emission phase (used by tools/, no runtime effect)
PHASE = [None]

# head processing order: evens (K-offset 0) then odds (K-offset 64) so
# concurrent different-K-offset score matmuls never share a PSUM bank
HEAD_ORDER = [0, 2, 4, 6, 1, 3, 5, 7]
# scores block of order-position idx lives at column 128*idx in the
# [96, 1024] scores tile (keeps each 96-col block inside one 2KB bank)
SCORE_COL = [128 * i for i in range(8)]
SUMS_COL = 992  # sums for real head h at column SUMS_COL + h (bank B)


def build_nc(apply_waitfix=True):
    nc = bass.Bass(trn_type="TRN2")
    x_d = nc.declare_dram_parameter("x", [4, 128, PIX], BF16, isOutput=False)
    wqk_d = nc.declare_dram_parameter("wqk", [4, 128, 1024], BF16, isOutput=False)
    wv_d = nc.declare_dram_parameter("wv", [4, 128, 512], BF16, isOutput=False)
    wo_d = nc.declare_dram_parameter("wo", [4, 128, 512], BF16, isOutput=False)
    bias_d = nc.declare_dram_parameter("bias", [4, 128, 1], F32, isOutput=False)
    ident_d = nc.declare_dram_parameter("ident", [96, 96], BF16, isOutput=False)
    y_d = nc.declare_dram_parameter("y", [512, PIX], F32, isOutput=True)

    with tile.TileContext(nc) as tc:
        with (
            tc.tile_pool(name="persist", bufs=1) as persist,
            tc.tile_pool(name="vrow", bufs=4) as vrow,
            tc.tile_pool(name="erow", bufs=4) as erow,
            tc.tile_pool(name="arow", bufs=4) as arow,
            tc.tile_pool(name="rrow", bufs=4) as rrow,
            tc.tile_pool(name="ostage", bufs=2) as ostage,
            tc.tile_pool(name="psA", bufs=2, space="PSUM") as psA,
            tc.tile_pool(name="psS", bufs=1, space="PSUM") as psS,
            tc.tile_pool(name="psV", bufs=1, space="PSUM") as psV,
            tc.tile_pool(name="psT", bufs=1, space="PSUM") as psT,
            tc.tile_pool(name="psSum", bufs=1, space="PSUM") as psSum,
            tc.tile_pool(name="psO", bufs=1, space="PSUM") as psO,
        ):
            # --- PE warmup: dependency-free dummy matmuls fill the
            # initial DMA wait and finish the clock ramp -----------------
            warm_sb = persist.tile([128, 512], BF16, tag="warm")
            nc.vector.memset(warm_sb[:, 0:128], 0.0)
            wps = psA.tile([128, 512], F32, tag="psA")
            for _ in range(4):
                nc.tensor.matmul(wps[:, 0:128], lhsT=warm_sb[:, 0:128],
                                 rhs=warm_sb[:, 0:128])
            nc.vector.memset(warm_sb[:, 128:512], 0.0)
            for _ in range(9):
                nc.tensor.matmul(wps[:, 0:512], lhsT=warm_sb[:, 0:128],
                                 rhs=warm_sb[:, :])

            # --- persistent loads (wqk + first x tiles first so the
            # projection GEMMs start as early as possible) --------------
            wqk_t, wv_t, wo_t, bias_t = [], [], [], []
            for cc in range(4):
                wt = persist.tile([128, 1024], BF16, name=f"wqk{cc}",
                                  tag=f"wqk{cc}")
                # split across the SWDGE (Pool) and HWDGE (SP) dispatch
                # paths so all four transfers beat the secondary loads
                eng = nc.gpsimd if cc < 2 else nc.sync
                eng.dma_start(out=wt[:, :], in_=wqk_d[cc])
                wqk_t.append(wt)
            # x in growing chunks: early groups land fast, later ones
            # amortize the ~0.65us per-DMA sequencer dispatch cost.
            # Only chunk 0 is issued here -- everything else is deferred
            # until after the prologue q/k chunks so the critical wqk
            # transfers are not queued behind secondary loads.
            CHUNKS = [(0, 1), (1, 2), (2, 4), (4, 6), (6, 9), (9, 12)]
            x_t = [[None] * GRP for _ in range(4)]

            def load_x_chunk(ci):
                t0, t1 = CHUNKS[ci]
                w = (t1 - t0) * GPIX
                for cc in range(4):
                    xt = persist.tile([128, w], BF16, name=f"x{cc}_c{ci}",
                                      tag=f"x{cc}_c{ci}")
                    nc.sync.dma_start(
                        out=xt[:, :],
                        in_=x_d[cc][:, t0 * GPIX:t1 * GPIX])
                    for t in range(t0, t1):
                        x_t[cc][t] = xt[:, (t - t0) * GPIX:(t - t0 + 1) * GPIX]

            load_x_chunk(0)
            ones_t = persist.tile([96, 1], BF16, tag="ones")
            nc.vector.memset(ones_t[:, :], 1.0)

            def load_secondary():
                load_x_chunk(1)
                for cc in range(4):
                    vt = persist.tile([128, 512], BF16, name=f"wv{cc}",
                                      tag=f"wv{cc}")
                    nc.sync.dma_start(out=vt[:, :], in_=wv_d[cc])
                    wv_t.append(vt)
                ident_t = persist.tile([96, 96], BF16, tag="ident")
                nc.sync.dma_start(out=ident_t[:, :], in_=ident_d[:, :])
                load_x_chunk(2)
                for cc in range(4):
                    ot = persist.tile([128, 512], BF16, name=f"wo{cc}",
                                      tag=f"wo{cc}")
                    nc.sync.dma_start(out=ot[:, :], in_=wo_d[cc])
                    wo_t.append(ot)
                    bt = persist.tile([128, 1], F32, name=f"bias{cc}",
                                      tag=f"bias{cc}")
                    nc.sync.dma_start(out=bt[:, :], in_=bias_d[cc])
                    bias_t.append(bt)
                for ci in range(3, len(CHUNKS)):
                    load_x_chunk(ci)
                return ident_t

            qk_t = [[None] * GRP for _ in range(8)]   # [oc][t] bf16 [128,384]
            # per-row state tiles, indexed by slot
            v_sb = [None] * NROW      # [96, 512] bf16 pixel-major v
            e_sb = [None] * NROW      # [96, 768] bf16 expS (head-order-major)
            av_ps = [None] * NROW     # AV psum tile [96, 512]
            a_sb = [None] * NROW      # [96, 512] bf16 normalized attn out
            rc_sb = [None] * NROW     # [96, 8] f32 reciprocal sums
            a_cm = [None] * GRP       # [128, 4, 384] bf16 channel-major attn
            o_sb = [None] * GRP       # y staging [128, 4, 384] f32 per group

            def emit_qk_chunk(t, oc, pool=None, ptag="psA"):
                """One q/k projection output chunk [128, 384] for group t."""
                PHASE[0] = f"qk(t{t},oc{oc})"
                if pool is None:
                    pool = psA
                qps = pool.tile([128, GPIX], F32, tag=ptag)
                for cc in range(4):
                    nc.tensor.matmul(
                        qps[:, 0:GPIX],
                        lhsT=wqk_t[cc][:, oc * 128:(oc + 1) * 128],
                        rhs=x_t[cc][t][:, :],
                        start=(cc == 0), stop=(cc == 3),
                    )
                qt = persist.tile([128, GPIX], BF16, tag=f"qk{oc}_{t}")
                if oc % 2 == 0:
                    nc.scalar.copy(out=qt[:, :], in_=qps[:, 0:GPIX])
                else:
                    nc.vector.tensor_copy(out=qt[:, :], in_=qps[:, 0:GPIX])
                qk_t[oc][t] = qt

            def emit_front(s, scores_first=False):
                """v projection + scores + exp for row-slot s."""
                PHASE[0] = f"front({s})"
                t, rr = s // 4, s % 4
                rsl = slice(rr * 96, rr * 96 + 96)

                def do_vproj():
                    vps = psA.tile([128, 512], F32, tag="psA")
                    for cc in range(4):
                        nc.tensor.matmul(
                            vps[0:96, 0:512],
                            lhsT=x_t[cc][t][:, rsl],
                            rhs=wv_t[cc][:, :],
                            start=(cc == 0), stop=(cc == 3),
                        )
                    v_sb[s] = vrow.tile([96, 512], BF16, name="vsb", tag="vsb")
                    nc.vector.tensor_copy(out=v_sb[s][:, :],
                                          in_=vps[0:96, 0:512])

                def do_scores():
                    # scores^T per head: out[j, i] block at col 128*idx
                    sps = psS.tile([96, 1024], F32, tag="psS")
                    for idx, h in enumerate(HEAD_ORDER):
                        qc, half = h // 2, 64 * (h % 2)
                        col = SCORE_COL[idx]
                        nc.tensor.matmul(
                            sps[0:96, col:col + 96],
                            lhsT=qk_t[4 + qc][t][half:half + 64, rsl],
                            rhs=qk_t[qc][t][half:half + 64, rsl],
                        )
                    e_sb[s] = erow.tile([96, 768], BF16, name="esb", tag="esb")
                    nc.scalar.activation(
                        out=e_sb[s].rearrange("p (k n) -> p k n", k=8),
                        in_=sps[0:96, :].rearrange("p (k m) -> p k m", k=8)
                            [:, :, 0:96],
                        func=mybir.ActivationFunctionType.Exp,
                        scale=SCALE,
                    )

                if scores_first:
                    do_scores()
                    do_vproj()
                else:
                    do_vproj()
                    do_scores()

            def emit_mid(s):
                """AV (transposed form) + sums + reciprocal for slot s."""
                PHASE[0] = f"mid({s})"
                avp = psV.tile([96, 512], F32, tag="psV")
                av_ps[s] = avp
                smp = psSum.tile([96, 8], F32, tag="psSum")
                for idx, h in enumerate(HEAD_ORDER):
                    eh = e_sb[s][:, idx * 96:(idx + 1) * 96]
                    nc.tensor.matmul(
                        avp[0:96, h * 64:(h + 1) * 64],
                        lhsT=eh,
                        rhs=v_sb[s][:, h * 64:(h + 1) * 64],
                    )
                    nc.tensor.matmul(
                        smp[:, h:h + 1],
                        lhsT=eh,
                        rhs=ones_t[:, :],
                    )
                rc_sb[s] = rrow.tile([96, 8], F32, name="rcsb", tag="rcsb")
                nc.vector.reciprocal(out=rc_sb[s][:, :], in_=smp[:, :])

            def emit_back_norm(s):
                """normalize attention output for slot s (DVE only)."""
                PHASE[0] = f"back({s})"
                a_sb[s] = arow.tile([96, 512], BF16, name="asb", tag="asb")
                nc.vector.tensor_tensor(
                    out=a_sb[s].rearrange("p (h d) -> p h d", h=8),
                    in0=av_ps[s][0:96, :].rearrange("p (h d) -> p h d", h=8),
                    in1=rc_sb[s][:, :, None].broadcast_to([96, 8, 64]),
                    op=mybir.AluOpType.mult,
                )

            def emit_back_tp(s):
                """PE transpose to channel-major + evacuate for slot s."""
                PHASE[0] = f"back({s})"
                t, rr = s // 4, s % 4
                tp = psT.tile([128, 384], BF16, tag="psT")
                for cc in range(4):
                    nc.tensor.transpose(
                        out=tp[:, cc * 96:(cc + 1) * 96],
                        in_=a_sb[s][:, cc * 128:(cc + 1) * 128],
                        identity=ident_t[:, :],
                    )
                if a_cm[t] is None:
                    a_cm[t] = persist.tile([128, 4, GPIX], BF16, name=f"acm{t}", tag=f"acm{t}")
                nc.vector.tensor_copy(
                    out=a_cm[t][:, :, rr * 96:rr * 96 + 96],
                    in_=tp[:, :].rearrange("p (c n) -> p c n", c=4),
                )

            def emit_outproj_row(s):
                """out projection + bias for the 96 pixels of row-slot s."""
                PHASE[0] = f"outproj({s})"
                t, rr = s // 4, s % 4
                # epilogue rows: use the otherwise-idle psA pool so
                # consecutive tail rows don't serialize on psO release
                pool = psA if s >= NROW - 3 else psO
                ops_ = pool.tile([128, 384], F32, tag="psA" if s >= NROW - 3 else "psO")
                for oc in range(4):
                    for cc in range(4):
                        nc.tensor.matmul(
                            ops_[:, oc * 96:oc * 96 + 96],
                            lhsT=wo_t[cc][:, oc * 128:(oc + 1) * 128],
                            rhs=a_cm[t][:, cc, rr * 96:rr * 96 + 96],
                            start=(cc == 0), stop=(cc == 3),
                        )
                tail = t == GRP - 1
                if o_sb[t] is None:
                    o_sb[t] = ostage.tile([128, 4, GPIX], F32, name="osb", tag="osb")
                for oc in range(4):
                    dst = o_sb[t][:, oc, rr * 96:rr * 96 + 96]
                    src = ops_[:, oc * 96:oc * 96 + 96]
                    if tail and oc in (1, 3):
                        # tail: spread biases across ACT+DVE so psum frees fast
                        nc.vector.tensor_scalar_add(dst, src, bias_t[oc][:, :])
                    else:
                        nc.scalar.add(out=dst, in_=src, add=bias_t[oc][:, :])
                if rr == 3 and t < GRP - 2:
                    # one DMA per group: a single HWDGE dispatch covers all
                    # four output-channel blocks
                    nc.sync.dma_start(
                        out=y_d[0:512, t * GPIX:(t + 1) * GPIX]
                            .rearrange("(c p) n -> p c n", c=4),
                        in_=o_sb[t][:, :, :])
                elif rr == 3:
                    # last group: per-oc DMAs; DVE-biased blocks are ready
                    # first, so dispatch them first on the serial HWDGE
                    for oc in (1, 3, 0, 2):
                        nc.sync.dma_start(
                            out=y_d[oc * 128:(oc + 1) * 128,
                                    t * GPIX:(t + 1) * GPIX],
                            in_=o_sb[t][:, oc, :])

            # --- prologue: q/k projection for group 0 (rotate across the
            # idle psum pools so evacuation latency never gates the PE) ---
            _ppools = [(psA, "psA"), (psO, "psO"), (psV, "psV"),
                       (psA, "psA"), (psO, "psO"), (psV, "psV"),
                       (psA, "psA"), (psO, "psO")]
            for oc in range(8):
                emit_qk_chunk(0, oc, pool=_ppools[oc][0], ptag=_ppools[oc][1])
            with tc.tile_wait_until(0.003):
                ident_t = load_secondary()
            # --- uniform row-slot pipeline ------------------------------
            for s in range(NROW):
                t, i = s // 4, s % 4
                if s >= 2:
                    emit_back_norm(s - 2)
                emit_front(s, scores_first=(s == NROW - 1))
                if s >= 1:
                    emit_mid(s - 1)
                if s >= 2:
                    emit_back_tp(s - 2)
                if t + 1 < GRP:
                    emit_qk_chunk(t + 1, 2 * i)
                    emit_qk_chunk(t + 1, 2 * i + 1)
                if s >= 3:
                    emit_outproj_row(s - 3)
            # --- epilogue: tightest drain order -------------------------
            emit_back_norm(NROW - 2)
            emit_mid(NROW - 1)
            emit_back_tp(NROW - 2)
            emit_outproj_row(NROW - 3)
            emit_back_norm(NROW - 1)
            emit_back_tp(NROW - 1)
            emit_outproj_row(NROW - 2)
            emit_outproj_row(NROW - 1)

    if apply_waitfix:
        split_excess_waits(nc)
    return nc


# --- walrus workaround -------------------------------------------------
# The walrus build in this container rejects instructions carrying more
# than a small number of semaphore waits (1 for CTRL-queue NoOp/Drain).
# TileContext's exit drain can exceed that. Split: keep at most one wait
# on the original instruction and insert same-engine NoOps immediately
# before it, each carrying one of the excess waits.
def split_excess_waits(nc):
    import bass_rust
    n_split = 0
    for f in nc.m.functions:
        for blk in f.blocks:
            newlist = []
            changed = False
            for inst in blk.instructions:
                si = inst.sync_info
                w = list(si.on_wait) if si is not None else []
                if len(w) > 1:
                    *pre, last = w
                    for ci, wait in enumerate(pre):
                        nop = mybir.InstNoOp(
                            name=f"{inst.name}-wsplit{ci}", ins=[], outs=[])
                        nop.engine = inst.engine
                        nop.sync_info = bass_rust.SyncInfo(
                            on_update=[], on_wait=[wait])
                        newlist.append(nop)
                    inst.sync_info.on_wait = [last]
                    changed = True
                    n_split += 1
                newlist.append(inst)
            if changed:
                blk.instructions = newlist
    return n_split


def shard_inputs(x, w_qkv, w_out, b_out):
    """Full inputs -> list of 8 per-core input maps."""
    x = np.asarray(x, dtype=np.float32)
    w_qkv = np.asarray(w_qkv, dtype=np.float32)
    w_out = np.asarray(w_out, dtype=np.float32)
    b_out = np.asarray(b_out, dtype=np.float32)
    bf = ml_dtypes.bfloat16
    wqk = np.ascontiguousarray(w_qkv[:1024].T).astype(bf).reshape(4, 128, 1024)
    wv = np.ascontiguousarray(w_qkv[1024:].T).astype(bf).reshape(4, 128, 512)
    wo = np.ascontiguousarray(w_out.T).astype(bf).reshape(4, 128, 512)
    bias = b_out.astype(np.float32).reshape(4, 128, 1)
    ident = np.eye(96, dtype=bf)
    xb = x.astype(bf)  # [4, 512, 96, 96]
    in_maps = []
    for core in range(NCORES):
        b, half = core // 2, core % 2
        xs = np.ascontiguousarray(
            xb[b, :, half * RPC:(half + 1) * RPC, :]).reshape(4, 128, PIX)
        in_maps.append({"x": xs, "wqk": wqk, "wv": wv, "wo": wo,
                        "bias": bias, "ident": ident})
    return in_maps


def unshard_outputs(results):
    out = np.empty((B, C, H, W), np.float32)
    for core in range(NCORES):
        b, half = core // 2, core % 2
        out[core // 2, :, (core % 2) * RPC:((core % 2) + 1) * RPC, :] = (
            results[core]["y"].reshape(C, RPC, W))
    return out


_NC_CACHE = None


def kernel(x, w_qkv, w_out, b_out):
    global _NC_CACHE
    from concourse.bass_utils import run_bass_kernel_spmd
    if _NC_CACHE is None:
        _NC_CACHE = build_nc()
    in_maps = shard_inputs(x, w_qkv, w_out, b_out)
    res = run_bass_kernel_spmd(_NC_CACHE, in_maps, list(range(NCORES)))
    return unshard_outputs(res.results)


# revision 65
# speedup vs baseline: 1.0034x; 1.0002x over previous
"""Trainium2 Bass kernel for AxialAttention (attention along W axis).

Sharding: pure data-parallel over (B=4) x (H split in 2) = 8 shards, one
per NeuronCore. Attention mixes pixels only along W within a single
(b, head, h-row), so splitting H requires no collectives.

Per-core structure (shard = [C=512, 48 rows x 96 cols]), organized as a
uniform pipeline over 48 row-slots (1 slot = one 96-pixel attention row):

  slot s (row i of group t, groups = 4 rows = 384 pixels):
    1. v projection for row s (pixel-major out [96pix, 512ch])
    2. scores^T = k^T.T @ q^T per head -> [j, i] blocks in PSUM,
       exp via ACT (scale=0.125, no max subtraction: |s*scale| < 7)
    3. two q/k-projection chunks of group t+1 (spread over the 4 slots)
    4. AV for row s-1 in transposed form: lhsT = expS [j,i], rhs = v
       [j, d] -> out [i-pixel, d-channel], ap=64/head (vs 96 for the
       [d, i] form); softmax sums via ap-1 matmuls (rhs = ones[96,1])
    5. normalize on DVE with broadcast reciprocal -> a [96, 512] bf16
    6. PE-transpose a back to channel-major (4 x [96,128] -> [128,96])
    7. one out-projection row (s-3) + bias via ACT -> y staging
  PSUM->SBUF evacuations are spread across DVE and ACT (GPSIMD cannot
  touch PSUM); PE is the only near-saturated engine (~95%).
"""

import numpy as np
import ml_dtypes

import concourse.bass as bass
import concourse.tile as tile
from concourse import mybir

BF16 = mybir.dt.bfloat16
F32 = mybir.dt.float32

B, C, H, W = 4, 512, 96, 96
HEADS, D = 8, 64
SCALE = 0.125
NCORES = 8
RPC = H // 2          # 48 rows per core
PIX = RPC * W         # 4608 pixels per core
GRP = 12              # pixel groups
GPIX = PIX // GRP     # 384 pixels per group = 4 rows
NROW = RPC            # 48 row-slots

# diagnostic hook: current emission phase (used by tools/, no runtime effect)
PHASE = [None]

# head processing order: evens (K-offset 0) then odds (K-offset 64) so
# concurrent different-K-offset score matmuls never share a PSUM bank
HEAD_ORDER = [0, 2, 4, 6, 1, 3, 5, 7]
# scores block of order-position idx lives at column 128*idx in the
# [96, 1024] scores tile (keeps each 96-col block inside one 2KB bank)
SCORE_COL = [128 * i for i in range(8)]
SUMS_COL = 992  # sums for real head h at column SUMS_COL + h (bank B)


def build_nc(apply_waitfix=True):
    nc = bass.Bass(trn_type="TRN2")
    x_d = nc.declare_dram_parameter("x", [4, 128, PIX], BF16, isOutput=False)
    wqk_d = nc.declare_dram_parameter("wqk", [4, 128, 1024], BF16, isOutput=False)
    wv_d = nc.declare_dram_parameter("wv", [4, 128, 512], BF16, isOutput=False)
    wo_d = nc.declare_dram_parameter("wo", [4, 128, 512], BF16, isOutput=False)
    bias_d = nc.declare_dram_parameter("bias", [4, 128, 1], F32, isOutput=False)
    ident_d = nc.declare_dram_parameter("ident", [96, 96], BF16, isOutput=False)
    y_d = nc.declare_dram_parameter("y", [512, PIX], F32, isOutput=True)

    with tile.TileContext(nc) as tc:
        with (
            tc.tile_pool(name="persist", bufs=1) as persist,
            tc.tile_pool(name="vrow", bufs=4) as vrow,
            tc.tile_pool(name="erow", bufs=4) as erow,
            tc.tile_pool(name="arow", bufs=4) as arow,
            tc.tile_pool(name="rrow", bufs=4) as rrow,
            tc.tile_pool(name="ostage", bufs=2) as ostage,
            tc.tile_pool(name="psA", bufs=2, space="PSUM") as psA,
            tc.tile_pool(name="psS", bufs=1, space="PSUM") as psS,
            tc.tile_pool(name="psV", bufs=1, space="PSUM") as psV,
            tc.tile_pool(name="psT", bufs=1, space="PSUM") as psT,
            tc.tile_pool(name="psSum", bufs=1, space="PSUM") as psSum,
            tc.tile_pool(name="psO", bufs=1, space="PSUM") as psO,
        ):
            # --- PE warmup: dependency-free dummy matmuls fill the
            # initial DMA wait and finish the clock ramp -----------------
            warm_sb = persist.tile([128, 512], BF16, tag="warm")
            nc.vector.memset(warm_sb[:, 0:128], 0.0)
            wps = psA.tile([128, 512], F32, tag="psA")
            for _ in range(4):
                nc.tensor.matmul(wps[:, 0:128], lhsT=warm_sb[:, 0:128],
                                 rhs=warm_sb[:, 0:128])
            nc.vector.memset(warm_sb[:, 128:512], 0.0)
            for _ in range(6):
                nc.tensor.matmul(wps[:, 0:512], lhsT=warm_sb[:, 0:128],
                                 rhs=warm_sb[:, :])

            # --- persistent loads (wqk + first x tiles first so the
            # projection GEMMs start as early as possible) --------------
            wqk_t, wv_t, wo_t, bias_t = [], [], [], []
            for cc in range(4):
                wt = persist.tile([128, 1024], BF16, name=f"wqk{cc}",
                                  tag=f"wqk{cc}")
                # split across the SWDGE (Pool) and HWDGE (SP) dispatch
                # paths so all four transfers beat the secondary loads
                eng = nc.gpsimd if cc < 2 else nc.sync
                eng.dma_start(out=wt[:, :], in_=wqk_d[cc])
                wqk_t.append(wt)
            # x in growing chunks: early groups land fast, later ones
            # amortize the ~0.65us per-DMA sequencer dispatch cost.
            # Only chunk 0 is issued here -- everything else is deferred
            # until after the prologue q/k chunks so the critical wqk
            # transfers are not queued behind secondary loads.
            CHUNKS = [(0, 1), (1, 2), (2, 4), (4, 6), (6, 9), (9, 12)]
            x_t = [[None] * GRP for _ in range(4)]

            def load_x_chunk(ci):
                t0, t1 = CHUNKS[ci]
                w = (t1 - t0) * GPIX
                for cc in range(4):
                    xt = persist.tile([128, w], BF16, name=f"x{cc}_c{ci}",
                                      tag=f"x{cc}_c{ci}")
                    nc.sync.dma_start(
                        out=xt[:, :],
                        in_=x_d[cc][:, t0 * GPIX:t1 * GPIX])
                    for t in range(t0, t1):
                        x_t[cc][t] = xt[:, (t - t0) * GPIX:(t - t0 + 1) * GPIX]

            load_x_chunk(0)
            ones_t = persist.tile([96, 1], BF16, tag="ones")
            nc.vector.memset(ones_t[:, :], 1.0)

            def load_secondary():
                load_x_chunk(1)
                for cc in range(4):
                    vt = persist.tile([128, 512], BF16, name=f"wv{cc}",
                                      tag=f"wv{cc}")
                    nc.sync.dma_start(out=vt[:, :], in_=wv_d[cc])
                    wv_t.append(vt)
                ident_t = persist.tile([96, 96], BF16, tag="ident")
                nc.sync.dma_start(out=ident_t[:, :], in_=ident_d[:, :])
                load_x_chunk(2)
                for cc in range(4):
                    ot = persist.tile([128, 512], BF16, name=f"wo{cc}",
                                      tag=f"wo{cc}")
                    nc.sync.dma_start(out=ot[:, :], in_=wo_d[cc])
                    wo_t.append(ot)
                    bt = persist.tile([128, 1], F32, name=f"bias{cc}",
                                      tag=f"bias{cc}")
                    nc.sync.dma_start(out=bt[:, :], in_=bias_d[cc])
                    bias_t.append(bt)
                for ci in range(3, len(CHUNKS)):
                    load_x_chunk(ci)
                return ident_t

            qk_t = [[None] * GRP for _ in range(8)]   # [oc][t] bf16 [128,384]
            # per-row state tiles, indexed by slot
            v_sb = [None] * NROW      # [96, 512] bf16 pixel-major v
            e_sb = [None] * NROW      # [96, 768] bf16 expS (head-order-major)
            av_ps = [None] * NROW     # AV psum tile [96, 512]
            a_sb = [None] * NROW      # [96, 512] bf16 normalized attn out
            rc_sb = [None] * NROW     # [96, 8] f32 reciprocal sums
            a_cm = [None] * GRP       # [128, 4, 384] bf16 channel-major attn
            o_sb = [None] * GRP       # y staging [128, 4, 384] f32 per group

            def emit_qk_chunk(t, oc, pool=None, ptag="psA"):
                """One q/k projection output chunk [128, 384] for group t."""
                PHASE[0] = f"qk(t{t},oc{oc})"
                if pool is None:
                    pool = psA
                qps = pool.tile([128, GPIX], F32, tag=ptag)
                for cc in range(4):
                    nc.tensor.matmul(
                        qps[:, 0:GPIX],
                        lhsT=wqk_t[cc][:, oc * 128:(oc + 1) * 128],
                        rhs=x_t[cc][t][:, :],
                        start=(cc == 0), stop=(cc == 3),
                    )
                qt = persist.tile([128, GPIX], BF16, tag=f"qk{oc}_{t}")
                if oc % 2 == 0:
                    nc.scalar.copy(out=qt[:, :], in_=qps[:, 0:GPIX])
                else:
                    nc.vector.tensor_copy(out=qt[:, :], in_=qps[:, 0:GPIX])
                qk_t[oc][t] = qt

            def emit_front(s, scores_first=False):
                """v projection + scores + exp for row-slot s."""
                PHASE[0] = f"front({s})"
                t, rr = s // 4, s % 4
                rsl = slice(rr * 96, rr * 96 + 96)

                def do_vproj():
                    vps = psA.tile([128, 512], F32, tag="psA")
                    for cc in range(4):
                        nc.tensor.matmul(
                            vps[0:96, 0:512],
                            lhsT=x_t[cc][t][:, rsl],
                            rhs=wv_t[cc][:, :],
                            start=(cc == 0), stop=(cc == 3),
                        )
                    v_sb[s] = vrow.tile([96, 512], BF16, name="vsb", tag="vsb")
                    nc.vector.tensor_copy(out=v_sb[s][:, :],
                                          in_=vps[0:96, 0:512])

                def do_scores():
                    # scores^T per head: out[j, i] block at col 128*idx
                    sps = psS.tile([96, 1024], F32, tag="psS")
                    for idx, h in enumerate(HEAD_ORDER):
                        qc, half = h // 2, 64 * (h % 2)
                        col = SCORE_COL[idx]
                        nc.tensor.matmul(
                            sps[0:96, col:col + 96],
                            lhsT=qk_t[4 + qc][t][half:half + 64, rsl],
                            rhs=qk_t[qc][t][half:half + 64, rsl],
                        )
                    e_sb[s] = erow.tile([96, 768], BF16, name="esb", tag="esb")
                    nc.scalar.activation(
                        out=e_sb[s].rearrange("p (k n) -> p k n", k=8),
                        in_=sps[0:96, :].rearrange("p (k m) -> p k m", k=8)
                            [:, :, 0:96],
                        func=mybir.ActivationFunctionType.Exp,
                        scale=SCALE,
                    )

                if scores_first:
                    do_scores()
                    do_vproj()
                else:
                    do_vproj()
                    do_scores()

            def emit_mid(s):
                """AV (transposed form) + sums + reciprocal for slot s."""
                PHASE[0] = f"mid({s})"
                avp = psV.tile([96, 512], F32, tag="psV")
                av_ps[s] = avp
                smp = psSum.tile([96, 8], F32, tag="psSum")
                for idx, h in enumerate(HEAD_ORDER):
                    eh = e_sb[s][:, idx * 96:(idx + 1) * 96]
                    nc.tensor.matmul(
                        avp[0:96, h * 64:(h + 1) * 64],
                        lhsT=eh,
                        rhs=v_sb[s][:, h * 64:(h + 1) * 64],
                    )
                    nc.tensor.matmul(
                        smp[:, h:h + 1],
                        lhsT=eh,
                        rhs=ones_t[:, :],
                    )
                rc_sb[s] = rrow.tile([96, 8], F32, name="rcsb", tag="rcsb")
                nc.vector.reciprocal(out=rc_sb[s][:, :], in_=smp[:, :])

            def emit_back_norm(s):
                """normalize attention output for slot s (DVE only)."""
                PHASE[0] = f"back({s})"
                a_sb[s] = arow.tile([96, 512], BF16, name="asb", tag="asb")
                nc.vector.tensor_tensor(
                    out=a_sb[s].rearrange("p (h d) -> p h d", h=8),
                    in0=av_ps[s][0:96, :].rearrange("p (h d) -> p h d", h=8),
                    in1=rc_sb[s][:, :, None].broadcast_to([96, 8, 64]),
                    op=mybir.AluOpType.mult,
                )

            def emit_back_tp(s):
                """PE transpose to channel-major + evacuate for slot s."""
                PHASE[0] = f"back({s})"
                t, rr = s // 4, s % 4
                tp = psT.tile([128, 384], BF16, tag="psT")
                for cc in range(4):
                    nc.tensor.transpose(
                        out=tp[:, cc * 96:(cc + 1) * 96],
                        in_=a_sb[s][:, cc * 128:(cc + 1) * 128],
                        identity=ident_t[:, :],
                    )
                if a_cm[t] is None:
                    a_cm[t] = persist.tile([128, 4, GPIX], BF16, name=f"acm{t}", tag=f"acm{t}")
                nc.vector.tensor_copy(
                    out=a_cm[t][:, :, rr * 96:rr * 96 + 96],
                    in_=tp[:, :].rearrange("p (c n) -> p c n", c=4),
                )

            def emit_outproj_row(s):
                """out projection + bias for the 96 pixels of row-slot s."""
                PHASE[0] = f"outproj({s})"
                t, rr = s // 4, s % 4
                # epilogue rows: use the otherwise-idle psA pool so
                # consecutive tail rows don't serialize on psO release
                pool = psA if s >= NROW - 3 else psO
                ops_ = pool.tile([128, 384], F32, tag="psA" if s >= NROW - 3 else "psO")
                for oc in range(4):
                    for cc in range(4):
                        nc.tensor.matmul(
                            ops_[:, oc * 96:oc * 96 + 96],
                            lhsT=wo_t[cc][:, oc * 128:(oc + 1) * 128],
                            rhs=a_cm[t][:, cc, rr * 96:rr * 96 + 96],
                            start=(cc == 0), stop=(cc == 3),
                        )
                tail = t == GRP - 1
                if o_sb[t] is None:
                    o_sb[t] = ostage.tile([128, 4, GPIX], F32, name="osb", tag="osb")
                for oc in range(4):
                    dst = o_sb[t][:, oc, rr * 96:rr * 96 + 96]
                    src = ops_[:, oc * 96:oc * 96 + 96]
                    if tail and oc in (1, 3):
                        # tail: spread biases across ACT+DVE so psum frees fast
                        nc.vector.tensor_scalar_add(dst, src, bias_t[oc][:, :])
                    else:
                        nc.scalar.add(out=dst, in_=src, add=bias_t[oc][:, :])
                if rr == 3 and t < GRP - 2:
                    # one DMA per group: a single HWDGE dispatch covers all
                    # four output-channel blocks
                    nc.sync.dma_start(
                        out=y_d[0:512, t * GPIX:(t + 1) * GPIX]
                            .rearrange("(c p) n -> p c n", c=4),
                        in_=o_sb[t][:, :, :])
                elif rr == 3:
                    # last group: per-oc DMAs; DVE-biased blocks are ready
                    # first, so dispatch them first on the serial HWDGE
                    for oc in (1, 3, 0, 2):
                        nc.sync.dma_start(
                            out=y_d[oc * 128:(oc + 1) * 128,
                                    t * GPIX:(t + 1) * GPIX],
                            in_=o_sb[t][:, oc, :])

            # --- prologue: q/k projection for group 0 (rotate across the
            # idle psum pools so evacuation latency never gates the PE) ---
            _ppools = [(psA, "psA"), (psO, "psO"), (psV, "psV"),
                       (psA, "psA"), (psO, "psO"), (psV, "psV"),
                       (psA, "psA"), (psO, "psO")]
            for oc in range(8):
                emit_qk_chunk(0, oc, pool=_ppools[oc][0], ptag=_ppools[oc][1])
            with tc.tile_wait_until(0.003):
                ident_t = load_secondary()
            # --- uniform row-slot pipeline ------------------------------
            for s in range(NROW):
                t, i = s // 4, s % 4
                if s >= 2:
                    emit_back_norm(s - 2)
                emit_front(s, scores_first=(s == NROW - 1))
                if s >= 1:
                    emit_mid(s - 1)
                if s >= 2:
                    emit_back_tp(s - 2)
                if t + 1 < GRP:
                    emit_qk_chunk(t + 1, 2 * i)
                    emit_qk_chunk(t + 1, 2 * i + 1)
                if s >= 3:
                    emit_outproj_row(s - 3)
            # --- epilogue: tightest drain order -------------------------
            emit_back_norm(NROW - 2)
            emit_mid(NROW - 1)
            emit_back_tp(NROW - 2)
            emit_outproj_row(NROW - 3)
            emit_back_norm(NROW - 1)
            emit_back_tp(NROW - 1)
            emit_outproj_row(NROW - 2)
            emit_outproj_row(NROW - 1)

    if apply_waitfix:
        split_excess_waits(nc)
    return nc


# --- walrus workaround -------------------------------------------------
# The walrus build in this container rejects instructions carrying more
# than a small number of semaphore waits (1 for CTRL-queue NoOp/Drain).
# TileContext's exit drain can exceed that. Split: keep at most one wait
# on the original instruction and insert same-engine NoOps immediately
# before it, each carrying one of the excess waits.
def split_excess_waits(nc):
    import bass_rust
    n_split = 0
    for f in nc.m.functions:
        for blk in f.blocks:
            newlist = []
            changed = False
            for inst in blk.instructions:
                si = inst.sync_info
                w = list(si.on_wait) if si is not None else []
                if len(w) > 1:
                    *pre, last = w
                    for ci, wait in enumerate(pre):
                        nop = mybir.InstNoOp(
                            name=f"{inst.name}-wsplit{ci}", ins=[], outs=[])
                        nop.engine = inst.engine
                        nop.sync_info = bass_rust.SyncInfo(
                            on_update=[], on_wait=[wait])
                        newlist.append(nop)
                    inst.sync_info.on_wait = [last]
                    changed = True
                    n_split += 1
                newlist.append(inst)
            if changed:
                blk.instructions = newlist
    return n_split


def shard_inputs(x, w_qkv, w_out, b_out):
    """Full inputs -> list of 8 per-core input maps."""
    x = np.asarray(x, dtype=np.float32)
    w_qkv = np.asarray(w_qkv, dtype=np.float32)
    w_out = np.asarray(w_out, dtype=np.float32)
    b_out = np.asarray(b_out, dtype=np.float32)
    bf = ml_dtypes.bfloat16
    wqk = np.ascontiguousarray(w_qkv[:1024].T).astype(bf).reshape(4, 128, 1024)
    wv = np.ascontiguousarray(w_qkv[1024:].T).astype(bf).reshape(4, 128, 512)
    wo = np.ascontiguousarray(w_out.T).astype(bf).reshape(4, 128, 512)
    bias = b_out.astype(np.float32).reshape(4, 128, 1)
    ident = np.eye(96, dtype=bf)
    xb = x.astype(bf)  # [4, 512, 96, 96]
    in_maps = []
    for core in range(NCORES):
        b, half = core // 2, core % 2
        xs = np.ascontiguousarray(
            xb[b, :, half * RPC:(half + 1) * RPC, :]).reshape(4, 128, PIX)
        in_maps.append({"x": xs, "wqk": wqk, "wv": wv, "wo": wo,
                        "bias": bias, "ident": ident})
    return in_maps


def unshard_outputs(results):
    out = np.empty((B, C, H, W), np.float32)
    for core in range(NCORES):
        b, half = core // 2, core % 2
        out[core // 2, :, (core % 2) * RPC:((core % 2) + 1) * RPC, :] = (
            results[core]["y"].reshape(C, RPC, W))
    return out


_NC_CACHE = None


def kernel(x, w_qkv, w_out, b_out):
    global _NC_CACHE
    from concourse.bass_utils import run_bass_kernel_spmd
    if _NC_CACHE is None:
        _NC_CACHE = build_nc()
    in_maps = shard_inputs(x, w_qkv, w_out, b_out)
    res = run_bass_kernel_spmd(_NC_CACHE, in_maps, list(range(NCORES)))
    return unshard_outputs(res.results)


# revision 74
# speedup vs baseline: 1.0054x; 1.0020x over previous
"""Trainium2 Bass kernel for AxialAttention (attention along W axis).

Sharding: pure data-parallel over (B=4) x (H split in 2) = 8 shards, one
per NeuronCore. Attention mixes pixels only along W within a single
(b, head, h-row), so splitting H requires no collectives.

Per-core structure (shard = [C=512, 48 rows x 96 cols]), organized as a
uniform pipeline over 48 row-slots (1 slot = one 96-pixel attention row):

  slot s (row i of group t, groups = 4 rows = 384 pixels):
    1. v projection for row s (pixel-major out [96pix, 512ch])
    2. scores^T = k^T.T @ q^T per head -> [j, i] blocks in PSUM,
       exp via ACT (scale=0.125, no max subtraction: |s*scale| < 7)
    3. two q/k-projection chunks of group t+1 (spread over the 4 slots)
    4. AV for row s-1 in transposed form: lhsT = expS [j,i], rhs = v
       [j, d] -> out [i-pixel, d-channel], ap=64/head (vs 96 for the
       [d, i] form); softmax sums via ap-1 matmuls (rhs = ones[96,1])
    5. normalize on DVE with broadcast reciprocal -> a [96, 512] bf16
    6. PE-transpose a back to channel-major (4 x [96,128] -> [128,96])
    7. one out-projection row (s-3) + bias via ACT -> y staging
  PSUM->SBUF evacuations are spread across DVE and ACT (GPSIMD cannot
  touch PSUM); PE is the only near-saturated engine (~95%).
"""

import numpy as np
import ml_dtypes

import concourse.bass as bass
import concourse.tile as tile
from concourse import mybir

BF16 = mybir.dt.bfloat16
F32 = mybir.dt.float32

B, C, H, W = 4, 512, 96, 96
HEADS, D = 8, 64
SCALE = 0.125
NCORES = 8
RPC = H // 2          # 48 rows per core
PIX = RPC * W         # 4608 pixels per core
GRP = 12              # pixel groups
GPIX = PIX // GRP     # 384 pixels per group = 4 rows
NROW = RPC            # 48 row-slots

# diagnostic hook: current emission phase (used by tools/, no runtime effect)
PHASE = [None]

# head processing order: evens (K-offset 0) then odds (K-offset 64) so
# concurrent different-K-offset score matmuls never share a PSUM bank
HEAD_ORDER = [0, 2, 4, 6, 1, 3, 5, 7]
# scores block of order-position idx lives at column 128*idx in the
# [96, 1024] scores tile (keeps each 96-col block inside one 2KB bank)
SCORE_COL = [128 * i for i in range(8)]
SUMS_COL = 992  # sums for real head h at column SUMS_COL + h (bank B)


def build_nc(apply_waitfix=True):
    nc = bass.Bass(trn_type="TRN2")
    x_d = nc.declare_dram_parameter("x", [4, 128, PIX], BF16, isOutput=False)
    wqk_d = nc.declare_dram_parameter("wqk", [4, 128, 1024], BF16, isOutput=False)
    wv_d = nc.declare_dram_parameter("wv", [4, 128, 512], BF16, isOutput=False)
    wo_d = nc.declare_dram_parameter("wo", [4, 128, 512], BF16, isOutput=False)
    bias_d = nc.declare_dram_parameter("bias", [4, 128, 1], F32, isOutput=False)
    ident_d = nc.declare_dram_parameter("ident", [96, 96], BF16, isOutput=False)
    y_d = nc.declare_dram_parameter("y", [512, PIX], F32, isOutput=True)

    with tile.TileContext(nc) as tc:
        with (
            tc.tile_pool(name="persist", bufs=1) as persist,
            tc.tile_pool(name="vrow", bufs=4) as vrow,
            tc.tile_pool(name="erow", bufs=4) as erow,
            tc.tile_pool(name="arow", bufs=4) as arow,
            tc.tile_pool(name="rrow", bufs=4) as rrow,
            tc.tile_pool(name="ostage", bufs=2) as ostage,
            tc.tile_pool(name="psA", bufs=2, space="PSUM") as psA,
            tc.tile_pool(name="psS", bufs=1, space="PSUM") as psS,
            tc.tile_pool(name="psV", bufs=1, space="PSUM") as psV,
            tc.tile_pool(name="psT", bufs=1, space="PSUM") as psT,
            tc.tile_pool(name="psSum", bufs=1, space="PSUM") as psSum,
            tc.tile_pool(name="psO", bufs=1, space="PSUM") as psO,
        ):
            # --- PE warmup: dependency-free dummy matmuls fill the
            # initial DMA wait and finish the clock ramp -----------------
            warm_sb = persist.tile([128, 512], BF16, tag="warm")
            nc.vector.memset(warm_sb[:, 0:128], 0.0)
            wps = psA.tile([128, 512], F32, tag="psA")
            for _ in range(4):
                nc.tensor.matmul(wps[:, 0:128], lhsT=warm_sb[:, 0:128],
                                 rhs=warm_sb[:, 0:128])
            nc.vector.memset(warm_sb[:, 128:512], 0.0)
            for _ in range(4):
                nc.tensor.matmul(wps[:, 0:512], lhsT=warm_sb[:, 0:128],
                                 rhs=warm_sb[:, :])

            # --- persistent loads (wqk + first x tiles first so the
            # projection GEMMs start as early as possible) --------------
            wv_t, wo_t, bias_t = [], [], []
            # wqk loaded as four oc-pair slices [128, 4cc, 256]: the first
            # q/k chains need only the first slice, so they start as soon
            # as it and the x chunk-0 pieces land
            wqk_p = []
            for p in range(4):
                wp = persist.tile([128, 4, 256], BF16, name=f"wqkp{p}",
                                  tag=f"wqkp{p}")
                wqk_p.append(wp)
            nc.gpsimd.dma_start(
                out=wqk_p[0][:, :, :],
                in_=wqk_d[:, :, 0:256].rearrange("c p n -> p c n"))
            nc.gpsimd.dma_start(
                out=wqk_p[1][:, :, :],
                in_=wqk_d[:, :, 256:512].rearrange("c p n -> p c n"))
            # x in growing chunks: early groups land fast, later ones
            # amortize the ~0.65us per-DMA sequencer dispatch cost.
            # Only chunk 0 is issued here -- everything else is deferred
            # until after the prologue q/k chunks so the critical wqk
            # transfers are not queued behind secondary loads.
            CHUNKS = [(0, 1), (1, 2), (2, 4), (4, 6), (6, 9), (9, 12)]
            x_t = [[None] * GRP for _ in range(4)]

            def load_x_chunk(ci):
                t0, t1 = CHUNKS[ci]
                w = (t1 - t0) * GPIX
                for cc in range(4):
                    xt = persist.tile([128, w], BF16, name=f"x{cc}_c{ci}",
                                      tag=f"x{cc}_c{ci}")
                    nc.sync.dma_start(
                        out=xt[:, :],
                        in_=x_d[cc][:, t0 * GPIX:t1 * GPIX])
                    for t in range(t0, t1):
                        x_t[cc][t] = xt[:, (t - t0) * GPIX:(t - t0 + 1) * GPIX]

            load_x_chunk(0)
            nc.sync.dma_start(
                out=wqk_p[2][:, :, :],
                in_=wqk_d[:, :, 512:768].rearrange("c p n -> p c n"))
            nc.sync.dma_start(
                out=wqk_p[3][:, :, :],
                in_=wqk_d[:, :, 768:1024].rearrange("c p n -> p c n"))
            ones_t = persist.tile([96, 1], BF16, tag="ones")
            nc.vector.memset(ones_t[:, :], 1.0)

            def load_secondary():
                load_x_chunk(1)
                for cc in range(4):
                    vt = persist.tile([128, 512], BF16, name=f"wv{cc}",
                                      tag=f"wv{cc}")
                    nc.sync.dma_start(out=vt[:, :], in_=wv_d[cc])
                    wv_t.append(vt)
                ident_t = persist.tile([96, 96], BF16, tag="ident")
                nc.sync.dma_start(out=ident_t[:, :], in_=ident_d[:, :])
                load_x_chunk(2)
                for cc in range(4):
                    ot = persist.tile([128, 512], BF16, name=f"wo{cc}",
                                      tag=f"wo{cc}")
                    nc.sync.dma_start(out=ot[:, :], in_=wo_d[cc])
                    wo_t.append(ot)
                    bt = persist.tile([128, 1], F32, name=f"bias{cc}",
                                      tag=f"bias{cc}")
                    nc.sync.dma_start(out=bt[:, :], in_=bias_d[cc])
                    bias_t.append(bt)
                for ci in range(3, len(CHUNKS)):
                    load_x_chunk(ci)
                return ident_t

            qk_t = [[None] * GRP for _ in range(8)]   # [oc][t] bf16 [128,384]
            # per-row state tiles, indexed by slot
            v_sb = [None] * NROW      # [96, 512] bf16 pixel-major v
            e_sb = [None] * NROW      # [96, 768] bf16 expS (head-order-major)
            av_ps = [None] * NROW     # AV psum tile [96, 512]
            a_sb = [None] * NROW      # [96, 512] bf16 normalized attn out
            rc_sb = [None] * NROW     # [96, 8] f32 reciprocal sums
            a_cm = [None] * GRP       # [128, 4, 384] bf16 channel-major attn
            o_sb = [None] * GRP       # y staging [128, 4, 384] f32 per group

            def emit_qk_chunk(t, oc, pool=None, ptag="psA"):
                """One q/k projection output chunk [128, 384] for group t."""
                PHASE[0] = f"qk(t{t},oc{oc})"
                if pool is None:
                    pool = psA
                qps = pool.tile([128, GPIX], F32, tag=ptag)
                for cc in range(4):
                    nc.tensor.matmul(
                        qps[:, 0:GPIX],
                        lhsT=wqk_p[oc // 2][:, cc,
                                            (oc % 2) * 128:(oc % 2) * 128 + 128],
                        rhs=x_t[cc][t][:, :],
                        start=(cc == 0), stop=(cc == 3),
                    )
                qt = persist.tile([128, GPIX], BF16, tag=f"qk{oc}_{t}")
                if oc % 2 == 0:
                    nc.scalar.copy(out=qt[:, :], in_=qps[:, 0:GPIX])
                else:
                    nc.vector.tensor_copy(out=qt[:, :], in_=qps[:, 0:GPIX])
                qk_t[oc][t] = qt

            def emit_front(s, scores_first=False):
                """v projection + scores + exp for row-slot s."""
                PHASE[0] = f"front({s})"
                t, rr = s // 4, s % 4
                rsl = slice(rr * 96, rr * 96 + 96)

                def do_vproj():
                    vps = psA.tile([128, 512], F32, tag="psA")
                    for cc in range(4):
                        nc.tensor.matmul(
                            vps[0:96, 0:512],
                            lhsT=x_t[cc][t][:, rsl],
                            rhs=wv_t[cc][:, :],
                            start=(cc == 0), stop=(cc == 3),
                        )
                    v_sb[s] = vrow.tile([96, 512], BF16, name="vsb", tag="vsb")
                    nc.vector.tensor_copy(out=v_sb[s][:, :],
                                          in_=vps[0:96, 0:512])

                def do_scores():
                    # scores^T per head: out[j, i] block at col 128*idx
                    sps = psS.tile([96, 1024], F32, tag="psS")
                    for idx, h in enumerate(HEAD_ORDER):
                        qc, half = h // 2, 64 * (h % 2)
                        col = SCORE_COL[idx]
                        nc.tensor.matmul(
                            sps[0:96, col:col + 96],
                            lhsT=qk_t[4 + qc][t][half:half + 64, rsl],
                            rhs=qk_t[qc][t][half:half + 64, rsl],
                        )
                    e_sb[s] = erow.tile([96, 768], BF16, name="esb", tag="esb")
                    nc.scalar.activation(
                        out=e_sb[s].rearrange("p (k n) -> p k n", k=8),
                        in_=sps[0:96, :].rearrange("p (k m) -> p k m", k=8)
                            [:, :, 0:96],
                        func=mybir.ActivationFunctionType.Exp,
                        scale=SCALE,
                    )

                if scores_first:
                    do_scores()
                    do_vproj()
                else:
                    do_vproj()
                    do_scores()

            def emit_mid(s):
                """AV (transposed form) + sums + reciprocal for slot s."""
                PHASE[0] = f"mid({s})"
                avp = psV.tile([96, 512], F32, tag="psV")
                av_ps[s] = avp
                smp = psSum.tile([96, 8], F32, tag="psSum")
                for idx, h in enumerate(HEAD_ORDER):
                    eh = e_sb[s][:, idx * 96:(idx + 1) * 96]
                    nc.tensor.matmul(
                        avp[0:96, h * 64:(h + 1) * 64],
                        lhsT=eh,
                        rhs=v_sb[s][:, h * 64:(h + 1) * 64],
                    )
                    nc.tensor.matmul(
                        smp[:, h:h + 1],
                        lhsT=eh,
                        rhs=ones_t[:, :],
                    )
                rc_sb[s] = rrow.tile([96, 8], F32, name="rcsb", tag="rcsb")
                nc.vector.reciprocal(out=rc_sb[s][:, :], in_=smp[:, :])

            def emit_back_norm(s):
                """normalize attention output for slot s (DVE only)."""
                PHASE[0] = f"back({s})"
                a_sb[s] = arow.tile([96, 512], BF16, name="asb", tag="asb")
                nc.vector.tensor_tensor(
                    out=a_sb[s].rearrange("p (h d) -> p h d", h=8),
                    in0=av_ps[s][0:96, :].rearrange("p (h d) -> p h d", h=8),
                    in1=rc_sb[s][:, :, None].broadcast_to([96, 8, 64]),
                    op=mybir.AluOpType.mult,
                )

            def emit_back_tp(s):
                """PE transpose to channel-major + evacuate for slot s."""
                PHASE[0] = f"back({s})"
                t, rr = s // 4, s % 4
                tp = psT.tile([128, 384], BF16, tag="psT")
                for cc in range(4):
                    nc.tensor.transpose(
                        out=tp[:, cc * 96:(cc + 1) * 96],
                        in_=a_sb[s][:, cc * 128:(cc + 1) * 128],
                        identity=ident_t[:, :],
                    )
                if a_cm[t] is None:
                    a_cm[t] = persist.tile([128, 4, GPIX], BF16, name=f"acm{t}", tag=f"acm{t}")
                nc.vector.tensor_copy(
                    out=a_cm[t][:, :, rr * 96:rr * 96 + 96],
                    in_=tp[:, :].rearrange("p (c n) -> p c n", c=4),
                )

            def emit_outproj_row(s):
                """out projection + bias for the 96 pixels of row-slot s."""
                PHASE[0] = f"outproj({s})"
                t, rr = s // 4, s % 4
                # epilogue rows: use the otherwise-idle psA pool so
                # consecutive tail rows don't serialize on psO release
                pool = psA if s >= NROW - 3 else psO
                ops_ = pool.tile([128, 384], F32, tag="psA" if s >= NROW - 3 else "psO")
                for oc in range(4):
                    for cc in range(4):
                        nc.tensor.matmul(
                            ops_[:, oc * 96:oc * 96 + 96],
                            lhsT=wo_t[cc][:, oc * 128:(oc + 1) * 128],
                            rhs=a_cm[t][:, cc, rr * 96:rr * 96 + 96],
                            start=(cc == 0), stop=(cc == 3),
                        )
                tail = t == GRP - 1
                if o_sb[t] is None:
                    o_sb[t] = ostage.tile([128, 4, GPIX], F32, name="osb", tag="osb")
                for oc in range(4):
                    dst = o_sb[t][:, oc, rr * 96:rr * 96 + 96]
                    src = ops_[:, oc * 96:oc * 96 + 96]
                    if tail and oc in (1, 3):
                        # tail: spread biases across ACT+DVE so psum frees fast
                        nc.vector.tensor_scalar_add(dst, src, bias_t[oc][:, :])
                    else:
                        nc.scalar.add(out=dst, in_=src, add=bias_t[oc][:, :])
                if rr == 3 and t < GRP - 2:
                    # one DMA per group: a single HWDGE dispatch covers all
                    # four output-channel blocks
                    nc.sync.dma_start(
                        out=y_d[0:512, t * GPIX:(t + 1) * GPIX]
                            .rearrange("(c p) n -> p c n", c=4),
                        in_=o_sb[t][:, :, :])
                elif rr == 3:
                    # last group: per-oc DMAs; DVE-biased blocks are ready
                    # first, so dispatch them first on the serial HWDGE
                    for oc in (1, 3, 0, 2):
                        nc.sync.dma_start(
                            out=y_d[oc * 128:(oc + 1) * 128,
                                    t * GPIX:(t + 1) * GPIX],
                            in_=o_sb[t][:, oc, :])

            # --- prologue: q/k projection for group 0 (rotate across the
            # idle psum pools so evacuation latency never gates the PE) ---
            _ppools = [(psA, "psA"), (psO, "psO"), (psV, "psV"),
                       (psA, "psA"), (psO, "psO"), (psV, "psV"),
                       (psA, "psA"), (psO, "psO")]
            for oc in range(8):
                emit_qk_chunk(0, oc, pool=_ppools[oc][0], ptag=_ppools[oc][1])
            with tc.tile_wait_until(0.003):
                ident_t = load_secondary()
            # --- uniform row-slot pipeline ------------------------------
            for s in range(NROW):
                t, i = s // 4, s % 4
                if s >= 2:
                    emit_back_norm(s - 2)
                emit_front(s, scores_first=(s == NROW - 1))
                if s >= 1:
                    emit_mid(s - 1)
                if s >= 2:
                    emit_back_tp(s - 2)
                if t + 1 < GRP:
                    emit_qk_chunk(t + 1, 2 * i)
                    emit_qk_chunk(t + 1, 2 * i + 1)
                if s >= 3:
                    emit_outproj_row(s - 3)
            # --- epilogue: tightest drain order -------------------------
            emit_back_norm(NROW - 2)
            emit_mid(NROW - 1)
            emit_back_tp(NROW - 2)
            emit_outproj_row(NROW - 3)
            emit_back_norm(NROW - 1)
            emit_back_tp(NROW - 1)
            emit_outproj_row(NROW - 2)
            emit_outproj_row(NROW - 1)

    if apply_waitfix:
        split_excess_waits(nc)
    return nc


# --- walrus workaround -------------------------------------------------
# The walrus build in this container rejects instructions carrying more
# than a small number of semaphore waits (1 for CTRL-queue NoOp/Drain).
# TileContext's exit drain can exceed that. Split: keep at most one wait
# on the original instruction and insert same-engine NoOps immediately
# before it, each carrying one of the excess waits.
def split_excess_waits(nc):
    import bass_rust
    n_split = 0
    for f in nc.m.functions:
        for blk in f.blocks:
            newlist = []
            changed = False
            for inst in blk.instructions:
                si = inst.sync_info
                w = list(si.on_wait) if si is not None else []
                if len(w) > 1:
                    *pre, last = w
                    for ci, wait in enumerate(pre):
                        nop = mybir.InstNoOp(
                            name=f"{inst.name}-wsplit{ci}", ins=[], outs=[])
                        nop.engine = inst.engine
                        nop.sync_info = bass_rust.SyncInfo(
                            on_update=[], on_wait=[wait])
                        newlist.append(nop)
                    inst.sync_info.on_wait = [last]
                    changed = True
                    n_split += 1
                newlist.append(inst)
            if changed:
                blk.instructions = newlist
    return n_split


def shard_inputs(x, w_qkv, w_out, b_out):
    """Full inputs -> list of 8 per-core input maps."""
    x = np.asarray(x, dtype=np.float32)
    w_qkv = np.asarray(w_qkv, dtype=np.float32)
    w_out = np.asarray(w_out, dtype=np.float32)
    b_out = np.asarray(b_out, dtype=np.float32)
    bf = ml_dtypes.bfloat16
    wqk = np.ascontiguousarray(w_qkv[:1024].T).astype(bf).reshape(4, 128, 1024)
    wv = np.ascontiguousarray(w_qkv[1024:].T).astype(bf).reshape(4, 128, 512)
    wo = np.ascontiguousarray(w_out.T).astype(bf).reshape(4, 128, 512)
    bias = b_out.astype(np.float32).reshape(4, 128, 1)
    ident = np.eye(96, dtype=bf)
    xb = x.astype(bf)  # [4, 512, 96, 96]
    in_maps = []
    for core in range(NCORES):
        b, half = core // 2, core % 2
        xs = np.ascontiguousarray(
            xb[b, :, half * RPC:(half + 1) * RPC, :]).reshape(4, 128, PIX)
        in_maps.append({"x": xs, "wqk": wqk, "wv": wv, "wo": wo,
                        "bias": bias, "ident": ident})
    return in_maps


def unshard_outputs(results):
    out = np.empty((B, C, H, W), np.float32)
    for core in range(NCORES):
        b, half = core // 2, core % 2
        out[core // 2, :, (core % 2) * RPC:((core % 2) + 1) * RPC, :] = (
            results[core]["y"].reshape(C, RPC, W))
    return out


_NC_CACHE = None


def kernel(x, w_qkv, w_out, b_out):
    global _NC_CACHE
    from concourse.bass_utils import run_bass_kernel_spmd
    if _NC_CACHE is None:
        _NC_CACHE = build_nc()
    in_maps = shard_inputs(x, w_qkv, w_out, b_out)
    res = run_bass_kernel_spmd(_NC_CACHE, in_maps, list(range(NCORES)))
    return unshard_outputs(res.results)
